# revision 2
# baseline (speedup 1.0000x reference)
"""DifferenceAwareAggregator v2 — Bass/Tile kernel, data-parallel on 8 cores.

Changes vs v1:
  * Neighbor COMPACTION: ~50% of neighbors are masked out; the host sorts
    centers by active-neighbor count, deals sorted 128-center tiles
    round-robin to cores (so all cores share one NEFF with identical
    per-slot neighbor counts L[j] = band max), and compacts each center's
    active neighbors to the front.  All per-neighbor work (pre/LN/Gelu/
    transpose/K/V/scores) shrinks to sum(L)/256 ~= 55%.
  * pre matmul in fp8e4 DoubleRow (2 k-tiles per instruction): hnf + w1s
    shipped as fp8; zb/Q/K/V stay bf16 (V-fp8 fails the 2e-2 gate).
  * Engine rebalance: Square pass on ActE (table-free), transpose-evac on
    Pool, at-broadcast fused into the pvt mul (drops the gpsimd ax copy),
    softmax batched per slot (one Exp table excursion).

Algebra: concat([h_n, h_n - h_c]) @ W1 == h_n @ (W1top+W1bot) - h_c @ W1bot.
bk drops out of softmax (per-(b,h) constant shift). 1/sqrt(d) folded into Wq.
"""

import sys

import numpy as np
import ml_dtypes

_TRN = "/opt/trn_rl_repo"
if _TRN not in sys.path:
    sys.path.insert(0, _TRN)

bf16 = ml_dtypes.bfloat16
f8 = ml_dtypes.float8_e4m3fn

M = 8          # cores
B = 8192
N = 32         # max neighbors
H = 512
BL = B // M    # centers per core
P = 128        # partitions
NS = BL // P   # center tiles (slots) per core
NHEAD = 8
DH = H // NHEAD
NEG = -30000.0
LN_EPS = 1e-5

_CACHE: dict = {}


def _plan(mask):
    """Sort centers by active count; deal sorted tiles round-robin to cores.

    Returns (order, L) where order[(8j+m)*128 + c] is the original center
    index at core m, slot j, lane c, and L[j] is slot j's neighbor count
    (max over its band of 8 tiles = count of the band's last center).
    """
    k = mask.sum(1).astype(np.int64)
    order = np.argsort(k, kind="stable")
    L = [int(k[order[(8 * j + 8) * P - 1]]) for j in range(NS)]
    return order, L


def _build_nc(L):
    import concourse.mybir as mybir
    import concourse.tile as tile
    from concourse import bacc
    from concourse.masks import make_identity

    f32 = mybir.dt.float32
    bf = mybir.dt.bfloat16
    fp8 = mybir.dt.float8e4
    Alu = mybir.AluOpType
    Act = mybir.ActivationFunctionType
    X = mybir.AxisListType.X
    DR = mybir.MatmulPerfMode.DoubleRow

    SL = sum(L)
    OFF = np.cumsum([0] + list(L))  # slot offsets into the packed n axis

    nc = bacc.Bacc()
    hnf_d = nc.dram_tensor("hnf", [H, SL, P], fp8, kind="ExternalInput")
    hcf_d = nc.dram_tensor("hcf", [H, BL], bf, kind="ExternalInput")
    mb_d = nc.dram_tensor("mb", [P, SL], f32, kind="ExternalInput")
    w1s_d = nc.dram_tensor("w1s", [H, H], fp8, kind="ExternalInput")
    w1b_d = nc.dram_tensor("w1b", [H, H], bf, kind="ExternalInput")
    wq_d = nc.dram_tensor("wq", [H, H], bf, kind="ExternalInput")
    wk_d = nc.dram_tensor("wk", [H, H], bf, kind="ExternalInput")
    wv_d = nc.dram_tensor("wv", [H, H], bf, kind="ExternalInput")
    wo_d = nc.dram_tensor("wo", [H, H], bf, kind="ExternalInput")
    out_d = nc.dram_tensor("out", [BL, H], bf, kind="ExternalOutput")

    # j = q*256 + two*128 + p  (DoubleRow pairs along the contraction dim)
    hnf_re = hnf_d.rearrange("(q two p) n c -> p q two n c", q=2, two=2, p=P)

    from contextlib import ExitStack

    with tile.TileContext(nc) as tc:
        with ExitStack() as stack:
            ec = stack.enter_context
            singles = ec(tc.tile_pool(name="singles", bufs=1))
            zbp = ec(tc.tile_pool(name="zb", bufs=2))
            hnin = ec(tc.tile_pool(name="hnin", bufs=5))
            tp = ec(tc.tile_pool(name="tp", bufs=2))
            sqs = ec(tc.tile_pool(name="sqs", bufs=2))
            hnp = ec(tc.tile_pool(name="hnp", bufs=3))
            stash = ec(tc.tile_pool(name="stash", bufs=2))
            pkp = ec(tc.tile_pool(name="pk", bufs=3))
            nwt = ec(tc.tile_pool(name="nwt", bufs=2))
            smx = ec(tc.tile_pool(name="smx", bufs=2))
            pvp = ec(tc.tile_pool(name="pv", bufs=3))
            tail = ec(tc.tile_pool(name="tail", bufs=2))
            mmps = ec(tc.tile_pool(name="mmps", bufs=4, space="PSUM"))
            trps = ec(tc.tile_pool(name="trps", bufs=2, space="PSUM"))
            ctxps = ec(tc.tile_pool(name="ctxps", bufs=2, space="PSUM"))

            # ---- persistent staging ----
            def load_w(dram_t):
                t = singles.tile([P, 4, H], bf, tag=f"w_{dram_t.name}")
                nc.sync.dma_start(out=t,
                                  in_=dram_t.rearrange("(fc p) j -> p fc j", p=P))
                return t

            hcf_t = singles.tile([P, 4, BL], bf, tag="hcf")
            nc.sync.dma_start(out=hcf_t,
                              in_=hcf_d.rearrange("(fc p) c -> p fc c", p=P))
            w1s_t = singles.tile([P, 2, 2, H], fp8, tag="w1s8")
            nc.sync.dma_start(
                out=w1s_t,
                in_=w1s_d.rearrange("(q two p) j -> p q two j", q=2, two=2, p=P))
            w1b_t = load_w(w1b_d)
            wq_t = load_w(wq_d)
            wk_t = wv_t = wo_t = mb_t = None

            identb = singles.tile([P, P], bf, tag="identb")
            make_identity(nc, identb)

            sumt = singles.tile([P, NS, N], f32, tag="sumt")
            sumsq = singles.tile([P, NS, N], f32, tag="sumsq")
            rs_all = singles.tile([P, NS, N], f32, tag="rs_all")
            nmurs = singles.tile([P, NS, N], f32, tag="nmurs")

            def groups(lj):
                g, n0 = [], 0
                while n0 < lj:
                    g.append((n0, min(4, lj - n0)))
                    n0 += min(4, lj - n0)
                return g

            def zq(j):
                zps = mmps.tile([P, H], f32, tag="mm")
                for fc in range(4):
                    nc.tensor.matmul(zps, hcf_t[:, fc, j * P:(j + 1) * P],
                                     w1b_t[:, fc], start=fc == 0, stop=fc == 3)
                zb = zbp.tile([P, H], f32, tag="zb")
                nc.scalar.copy(out=zb, in_=zps)
                qps = mmps.tile([P, H], f32, tag="mm")
                for fc in range(4):
                    nc.tensor.matmul(qps, hcf_t[:, fc, j * P:(j + 1) * P],
                                     wq_t[:, fc], start=fc == 0, stop=fc == 3)
                qs = zbp.tile([P, H], f32, tag="qs")
                nc.scalar.copy(out=qs, in_=qps)
                return zb, qs

            def stage1(j, zb):
                lj = L[j]
                t_j = tp.tile([P, lj, H], bf, tag="t")
                for n0, g in groups(lj):
                    hin = hnin.tile([P, 2, 2, g, P], fp8, tag="hnin")
                    nc.sync.dma_start(
                        out=hin,
                        in_=hnf_re[:, :, :, OFF[j] + n0:OFF[j] + n0 + g, :])
                    for k in range(g):
                        n = n0 + k
                        pre = mmps.tile([P, H], f32, tag="mm")
                        for hh in range(2):
                            for q in range(2):
                                nc.tensor.matmul(
                                    pre[:, hh * 256:(hh + 1) * 256],
                                    hin[:, q, :, k, :],
                                    w1s_t[:, q, :, hh * 256:(hh + 1) * 256],
                                    perf_mode=DR, start=q == 0, stop=q == 1)
                        nc.vector.scalar_tensor_tensor(
                            out=t_j[:, n], in0=pre, scalar=0.0, in1=zb,
                            op0=Alu.add, op1=Alu.subtract,
                            accum_out=sumt[:, j, n:n + 1])
                        sq = sqs.tile([P, H], bf, tag="sq")
                        nc.scalar.activation(out=sq, in_=t_j[:, n],
                                             func=Act.Square,
                                             accum_out=sumsq[:, j, n:n + 1])
                return t_j

            def newton(j):
                lj = L[j]
                sm = sumt[:, j, :lj]
                sq_ = sumsq[:, j, :lj]
                varH = nwt.tile([P, N], f32, tag="n_varH")
                a_t = nwt.tile([P, N], f32, tag="n_a")
                y_t = nwt.tile([P, N], f32, tag="n_y")
                u_t = nwt.tile([P, N], f32, tag="n_u")
                w_t = nwt.tile([P, N], f32, tag="n_w")
                musq = nwt.tile([P, N], f32, tag="n_musq")
                varH, a_t, y_t = varH[:, :lj], a_t[:, :lj], y_t[:, :lj]
                u_t, w_t, musq = u_t[:, :lj], w_t[:, :lj], musq[:, :lj]
                nc.vector.tensor_mul(musq, sm, sm)
                nc.vector.scalar_tensor_tensor(
                    out=varH, in0=musq, scalar=-1.0 / H, in1=sq_,
                    op0=Alu.mult, op1=Alu.add)
                nc.vector.tensor_scalar(out=a_t, in0=varH, scalar1=1.0 / H,
                                        scalar2=LN_EPS, op0=Alu.mult,
                                        op1=Alu.add)
                nc.vector.tensor_scalar(out=y_t, in0=a_t, scalar1=-1.35,
                                        scalar2=2.20, op0=Alu.mult,
                                        op1=Alu.add)
                nc.vector.tensor_scalar_max(y_t, y_t, 0.15)
                for _ in range(3):
                    nc.vector.tensor_mul(u_t, a_t, y_t)
                    nc.vector.tensor_mul(u_t, u_t, y_t)
                    nc.vector.tensor_scalar(out=w_t, in0=u_t, scalar1=-0.5,
                                            scalar2=1.5, op0=Alu.mult,
                                            op1=Alu.add)
                    nc.vector.tensor_mul(y_t, y_t, w_t)
                nc.vector.tensor_copy(rs_all[:, j, :lj], y_t)
                nc.vector.tensor_mul(u_t, sm, y_t)
                nc.vector.tensor_scalar_mul(nmurs[:, j, :lj], u_t, -1.0 / H)

            def stage2_softmax(j, qs, t_j):
                lj = L[j]
                hf_j = stash.tile([P, lj, 4, P], bf, tag="hf_stash")
                sc_j = smx.tile([P, NHEAD, lj], f32, tag="sc")
                for n in range(lj):
                    hn = hnp.tile([P, H], bf, tag="hn")
                    nc.scalar.activation(out=hn, in_=t_j[:, n],
                                         func=Act.Gelu,
                                         scale=rs_all[:, j, n:n + 1],
                                         bias=nmurs[:, j, n:n + 1])
                    tps = trps.tile([P, 4, P], bf, tag="tr")
                    for fc in range(4):
                        nc.tensor.transpose(tps[:, fc],
                                            hn[:, fc * P:(fc + 1) * P],
                                            identb)
                    nc.scalar.copy(out=hf_j[:, n], in_=tps)
                    kps = mmps.tile([P, H], f32, tag="mm")
                    for fc in range(4):
                        nc.tensor.matmul(kps, hf_j[:, n, fc], wk_t[:, fc],
                                         start=fc == 0, stop=fc == 3)
                    pkt = pkp.tile([P, H], f32, tag="pk")
                    nc.vector.tensor_mul(pkt, kps, qs)
                    nc.vector.reduce_sum(
                        out=sc_j[:, :, n],
                        in_=pkt.rearrange("c (h d) -> c h d", h=NHEAD),
                        axis=X)
                e_j = smx.tile([P, NHEAD, lj], f32, tag="e")
                at_j = smx.tile([P, NHEAD, lj], f32, tag="at")
                ssum = smx.tile([P, NHEAD], f32, tag="ssum")
                nc.vector.tensor_add(
                    e_j, sc_j,
                    mb_t[:, None, OFF[j]:OFF[j] + lj].to_broadcast(
                        (P, NHEAD, lj)))
                nc.scalar.activation(out=e_j, in_=e_j, func=Act.Exp)
                nc.vector.reduce_sum(out=ssum, in_=e_j, axis=X)
                nc.vector.reciprocal(ssum, ssum)
                nc.vector.tensor_mul(
                    at_j, e_j,
                    ssum[:, :, None].to_broadcast((P, NHEAD, lj)))
                return hf_j, at_j

            def stage3(j, hf_j, at_j):
                lj = L[j]
                ctx = ctxps.tile([P, H], f32, tag="ctx")
                for n in range(lj):
                    vps = mmps.tile([P, H], f32, tag="mm")
                    for fc in range(4):
                        nc.tensor.matmul(vps, hf_j[:, n, fc], wv_t[:, fc],
                                         start=fc == 0, stop=fc == 3)
                    pvt = pvp.tile([P, H], bf, tag="pv")
                    nc.vector.tensor_mul(
                        pvt.rearrange("c (h d) -> c h d", h=NHEAD),
                        vps.rearrange("c (h d) -> c h d", h=NHEAD),
                        at_j[:, :, n:n + 1].to_broadcast((P, NHEAD, DH)))
                    nc.tensor.matmul(ctx, identb, pvt,
                                     start=n == 0, stop=n == lj - 1)
                cs = tail.tile([P, H], bf, tag="cs")
                nc.vector.tensor_copy(out=cs, in_=ctx)
                cts = trps.tile([P, 4, P], bf, tag="tr")
                for fc in range(4):
                    nc.tensor.transpose(cts[:, fc], cs[:, fc * P:(fc + 1) * P],
                                        identb)
                ctf = tail.tile([P, 4, P], bf, tag="ctf")
                nc.vector.tensor_copy(out=ctf, in_=cts)
                ops = mmps.tile([P, H], f32, tag="mm")
                for fc in range(4):
                    nc.tensor.matmul(ops, ctf[:, fc], wo_t[:, fc],
                                     start=fc == 0, stop=fc == 3)
                ot = tail.tile([P, H], bf, tag="ot")
                nc.vector.tensor_copy(out=ot, in_=ops)
                nc.sync.dma_start(out=out_d[j * P:(j + 1) * P, :], in_=ot)

            # ---- software-pipelined slot loop ----
            zb_c, qs_c = zq(0)
            t_c = stage1(0, zb_c)
            wk_t = load_w(wk_d)
            wv_t = load_w(wv_d)
            wo_t = load_w(wo_d)
            mb_t = singles.tile([P, sum(L)], f32, tag="mb")
            nc.sync.dma_start(out=mb_t, in_=mb_d[:, :])
            newton(0)
            for j in range(NS):
                hf_c, at_c = stage2_softmax(j, qs_c, t_c)
                if j + 1 < NS:
                    zb_c, qs_c = zq(j + 1)
                    t_c = stage1(j + 1, zb_c)
                    newton(j + 1)
                stage3(j, hf_c, at_c)

    nc.finalize()
    return nc


def _get_nc(L=None):
    key = ("nc", tuple(L) if L else None)
    if key not in _CACHE:
        assert L is not None
        _CACHE[key] = _build_nc(list(L))
    return _CACHE[key]


def _pack_inputs(h_center, h_neighbors, neighbor_mask, W1, Wq, Wk, Wv, Wo):
    hn = np.asarray(h_neighbors, np.float32)
    hc = np.asarray(h_center, np.float32)
    mask = np.asarray(neighbor_mask)
    W1 = np.asarray(W1, np.float32)
    w1s = (W1[:H] + W1[H:]).astype(f8)
    w1b = W1[H:].astype(bf16)
    wq = (np.asarray(Wq, np.float32) / np.sqrt(DH)).astype(bf16)
    wk = np.asarray(Wk, bf16)
    wv = np.asarray(Wv, bf16)
    wo = np.asarray(Wo, bf16)

    order, L = _plan(mask)
    SL = sum(L)
    OFF = np.cumsum([0] + list(L))

    # per (core, slot): gather the 128 sorted centers, compact neighbors
    in_maps = []
    for m in range(M):
        hnf = np.zeros((H, SL, P), f8)
        hcf = np.empty((H, BL), np.float32)
        mb = np.full((P, SL), NEG, np.float32)
        for j in range(NS):
            gidx = order[(8 * j + m) * P:(8 * j + m) * P + P]
            hcf[:, j * P:(j + 1) * P] = hc[gidx].T
            for c, g in enumerate(gidx):
                act = np.nonzero(mask[g])[0]
                k = len(act)
                # [k, H] -> [H, k]
                hnf[:, OFF[j]:OFF[j] + k, c] = hn[g, act, :].T.astype(f8)
                mb[c, OFF[j]:OFF[j] + k] = 0.0
        in_maps.append({
            "hnf": np.ascontiguousarray(hnf),
            "hcf": np.ascontiguousarray(hcf.astype(bf16)),
            "mb": np.ascontiguousarray(mb),
            "w1s": w1s, "w1b": w1b, "wq": wq, "wk": wk, "wv": wv, "wo": wo,
        })
    return in_maps, order, L


def _fast_path_ok(b1, ln_g, ln_b, bq, bk, bv, bo):
    return (np.all(np.asarray(b1) == 0) and np.all(np.asarray(ln_g) == 1)
            and np.all(np.asarray(ln_b) == 0) and np.all(np.asarray(bq) == 0)
            and np.all(np.asarray(bv) == 0) and np.all(np.asarray(bo) == 0))


def _np_fallback(h_center, h_neighbors, W1, b1, ln_g, ln_b, Wq, bq, Wk, bk,
                 Wv, bv, Wo, bo, neighbor_mask):
    from scipy.special import erf

    hc = np.asarray(h_center, np.float32)
    hn = np.asarray(h_neighbors, np.float32)
    diff = hn - hc[:, None, :]
    comb = np.concatenate([hn, diff], -1)
    pre = comb @ W1 + b1
    mu = pre.mean(-1, keepdims=True)
    var = ((pre - mu) ** 2).mean(-1, keepdims=True)
    x = (pre - mu) / np.sqrt(var + LN_EPS) * ln_g + ln_b
    hnp_ = 0.5 * x * (1 + erf(x / np.sqrt(2)))
    Q = (hc @ Wq + bq).reshape(B, NHEAD, DH)
    K = (hnp_ @ Wk + bk).reshape(B, N, NHEAD, DH)
    V = (hnp_ @ Wv + bv).reshape(B, N, NHEAD, DH)
    sc = np.einsum("bhd,bnhd->bhn", Q, K) / np.sqrt(DH)
    sc = np.where(neighbor_mask[:, None, :], sc, -np.inf)
    sc = sc - sc.max(-1, keepdims=True)
    e = np.exp(sc)
    attn = e / e.sum(-1, keepdims=True)
    ctx = np.einsum("bhn,bnhd->bhd", attn, V).reshape(B, H)
    return (ctx @ Wo + bo).astype(np.float32)


def run_spmd(in_maps, L, **kwargs):
    from concourse.bass_utils import run_bass_kernel_spmd

    return run_bass_kernel_spmd(_get_nc(L), in_maps, core_ids=list(range(M)),
                                **kwargs)


def kernel(h_center, h_neighbors, W1, b1, ln_g, ln_b, Wq, bq, Wk, bk, Wv, bv,
           Wo, bo, neighbor_mask):
    if not _fast_path_ok(b1, ln_g, ln_b, bq, bk, bv, bo):
        return _np_fallback(h_center, h_neighbors, W1, b1, ln_g, ln_b, Wq, bq,
                            Wk, bk, Wv, bv, Wo, bo, neighbor_mask)
    in_maps, order, L = _pack_inputs(h_center, h_neighbors, neighbor_mask, W1,
                                     Wq, Wk, Wv, Wo)
    res = run_spmd(in_maps, L)
    dev_rows = np.concatenate(
        [np.asarray(r["out"], np.float32) for r in res.results], axis=0)
    # device row m*BL + j*P + c holds original center order[(8j+m)*P + c]
    out = np.empty((B, H), np.float32)
    out[_dev_perm(order)] = dev_rows
    return out


def _dev_perm(order):
    idx = np.empty(B, np.int64)
    for m in range(M):
        for j in range(NS):
            idx[m * BL + j * P:m * BL + (j + 1) * P] = \
                order[(8 * j + m) * P:(8 * j + m + 1) * P]
    return idx


# revision 4
# speedup vs baseline: 2.3216x; 2.3216x over previous
"""DifferenceAwareAggregator v2 — Bass/Tile kernel, data-parallel on 8 cores.

Changes vs v1:
  * Neighbor COMPACTION: ~50% of neighbors are masked out; the host sorts
    centers by active-neighbor count, deals sorted 128-center tiles
    round-robin to cores (so all cores share one NEFF with identical
    per-slot neighbor counts L[j] = band max), and compacts each center's
    active neighbors to the front.  All per-neighbor work (pre/LN/Gelu/
    transpose/K/V/scores) shrinks to sum(L)/256 ~= 55%.
  * pre matmul in fp8e4 DoubleRow (2 k-tiles per instruction): hnf + w1s
    shipped as fp8; zb/Q/K/V stay bf16 (V-fp8 fails the 2e-2 gate).
  * Engine rebalance: Square pass on ActE (table-free), transpose-evac on
    Pool, at-broadcast fused into the pvt mul (drops the gpsimd ax copy),
    softmax batched per slot (one Exp table excursion).

Algebra: concat([h_n, h_n - h_c]) @ W1 == h_n @ (W1top+W1bot) - h_c @ W1bot.
bk drops out of softmax (per-(b,h) constant shift). 1/sqrt(d) folded into Wq.
"""

import sys

import numpy as np
import ml_dtypes

_TRN = "/opt/trn_rl_repo"
if _TRN not in sys.path:
    sys.path.insert(0, _TRN)

bf16 = ml_dtypes.bfloat16
f8 = ml_dtypes.float8_e4m3fn

M = 8          # cores
B = 8192
N = 32         # max neighbors
H = 512
BL = B // M    # centers per core
P = 128        # partitions
NS = BL // P   # center tiles (slots) per core
NHEAD = 8
DH = H // NHEAD
NEG = -30000.0
LN_EPS = 1e-5

_CACHE: dict = {}
# build-time tuning knobs (sim experiments); key them into the nc cache
_OPTS = {"hf_dve": False, "trps_bufs": 3, "hnp_bufs": 3, "mmps_bufs": 4,
         "ctx_bufs": 1, "pkp_bufs": 3, "pvp_bufs": 3}


def _plan(mask):
    """Sort centers by active count; deal sorted tiles round-robin to cores.

    Returns (order, L) where order[(8j+m)*128 + c] is the original center
    index at core m, slot j, lane c, and L[j] is slot j's neighbor count
    (max over its band of 8 tiles = count of the band's last center).
    """
    k = mask.sum(1).astype(np.int64)
    order = np.argsort(k, kind="stable")
    L = [int(k[order[(8 * j + 8) * P - 1]]) for j in range(NS)]
    return order, L


def _build_nc(L):
    import concourse.mybir as mybir
    import concourse.tile as tile
    from concourse import bacc
    from concourse.masks import make_identity

    f32 = mybir.dt.float32
    bf = mybir.dt.bfloat16
    fp8 = mybir.dt.float8e4
    Alu = mybir.AluOpType
    Act = mybir.ActivationFunctionType
    X = mybir.AxisListType.X
    DR = mybir.MatmulPerfMode.DoubleRow

    SL = sum(L)
    OFF = np.cumsum([0] + list(L))  # slot offsets into the packed n axis

    nc = bacc.Bacc()
    hnf_d = nc.dram_tensor("hnf", [H, SL, P], fp8, kind="ExternalInput")
    hcf_d = nc.dram_tensor("hcf", [H, BL], bf, kind="ExternalInput")
    mb_d = nc.dram_tensor("mb", [P, SL], f32, kind="ExternalInput")
    w1s_d = nc.dram_tensor("w1s", [H, H], fp8, kind="ExternalInput")
    w1b_d = nc.dram_tensor("w1b", [H, H], bf, kind="ExternalInput")
    wq_d = nc.dram_tensor("wq", [H, H], bf, kind="ExternalInput")
    wk_d = nc.dram_tensor("wk", [H, H], bf, kind="ExternalInput")
    wv_d = nc.dram_tensor("wv", [H, H], bf, kind="ExternalInput")
    wo_d = nc.dram_tensor("wo", [H, H], bf, kind="ExternalInput")
    out_d = nc.dram_tensor("out", [BL, H], bf, kind="ExternalOutput")

    # j = q*256 + two*128 + p  (DoubleRow pairs along the contraction dim)
    hnf_re = hnf_d.rearrange("(q two p) n c -> p q two n c", q=2, two=2, p=P)

    from contextlib import ExitStack

    with tile.TileContext(nc) as tc:
        with ExitStack() as stack:
            ec = stack.enter_context
            singles = ec(tc.tile_pool(name="singles", bufs=1))
            zbp = ec(tc.tile_pool(name="zb", bufs=2))
            hnin = ec(tc.tile_pool(name="hnin", bufs=5))
            tp = ec(tc.tile_pool(name="tp", bufs=2))
            sqs = ec(tc.tile_pool(name="sqs", bufs=2))
            hnp = ec(tc.tile_pool(name="hnp", bufs=_OPTS["hnp_bufs"]))
            stash = ec(tc.tile_pool(name="stash", bufs=2))
            pkp = ec(tc.tile_pool(name="pk", bufs=_OPTS["pkp_bufs"]))
            nwt = ec(tc.tile_pool(name="nwt", bufs=2))
            smx = ec(tc.tile_pool(name="smx", bufs=2))
            pvp = ec(tc.tile_pool(name="pv", bufs=_OPTS["pvp_bufs"]))
            tail = ec(tc.tile_pool(name="tail", bufs=2))
            mmps = ec(tc.tile_pool(name="mmps", bufs=_OPTS["mmps_bufs"], space="PSUM"))
            trps = ec(tc.tile_pool(name="trps", bufs=_OPTS["trps_bufs"], space="PSUM"))
            ctxps = ec(tc.tile_pool(name="ctxps", bufs=_OPTS["ctx_bufs"], space="PSUM"))

            # ---- persistent staging ----
            def load_w(dram_t):
                t = singles.tile([P, 4, H], bf, tag=f"w_{dram_t.name}")
                nc.sync.dma_start(out=t,
                                  in_=dram_t.rearrange("(fc p) j -> p fc j", p=P))
                return t

            hcf_t = w1s_t = w1b_t = wq_t = None
            wk_t = wv_t = wo_t = mb_t = None

            identb = singles.tile([P, P], bf, tag="identb")
            make_identity(nc, identb)

            sumt = singles.tile([P, NS, N], f32, tag="sumt")
            sumsq = singles.tile([P, NS, N], f32, tag="sumsq")
            rs_all = singles.tile([P, NS, N], f32, tag="rs_all")
            nmurs = singles.tile([P, NS, N], f32, tag="nmurs")

            def groups(lj):
                g, n0 = [], 0
                while n0 < lj:
                    g.append((n0, min(4, lj - n0)))
                    n0 += min(4, lj - n0)
                return g

            def zq(j):
                zps = mmps.tile([P, H], f32, tag="mm")
                for fc in range(4):
                    nc.tensor.matmul(zps, hcf_t[:, fc, j * P:(j + 1) * P],
                                     w1b_t[:, fc], start=fc == 0, stop=fc == 3)
                zb = zbp.tile([P, H], f32, tag="zb")
                nc.scalar.copy(out=zb, in_=zps)
                qps = mmps.tile([P, H], f32, tag="mm")
                for fc in range(4):
                    nc.tensor.matmul(qps, hcf_t[:, fc, j * P:(j + 1) * P],
                                     wq_t[:, fc], start=fc == 0, stop=fc == 3)
                qs = zbp.tile([P, H], f32, tag="qs")
                nc.scalar.copy(out=qs, in_=qps)
                return zb, qs

            def issue_hin(j):
                tiles = []
                for n0, g in groups(L[j]):
                    hin = hnin.tile([P, 2, 2, g, P], fp8, tag="hnin")
                    nc.sync.dma_start(
                        out=hin,
                        in_=hnf_re[:, :, :, OFF[j] + n0:OFF[j] + n0 + g, :])
                    tiles.append(hin)
                return tiles

            def s1_n(j, n, zb, t_j, hin, k):
                pre = mmps.tile([P, H], f32, tag="mm")
                for hh in range(2):
                    for q in range(2):
                        nc.tensor.matmul(
                            pre[:, hh * 256:(hh + 1) * 256],
                            hin[:, q, :, k, :],
                            w1s_t[:, q, :, hh * 256:(hh + 1) * 256],
                            perf_mode=DR, start=q == 0, stop=q == 1)
                nc.vector.scalar_tensor_tensor(
                    out=t_j[:, n], in0=pre, scalar=0.0, in1=zb,
                    op0=Alu.add, op1=Alu.subtract,
                    accum_out=sumt[:, j, n:n + 1])
                sq = sqs.tile([P, H], bf, tag="sq")
                nc.scalar.activation(out=sq, in_=t_j[:, n],
                                     func=Act.Square,
                                     accum_out=sumsq[:, j, n:n + 1])

            def stage1(j, zb, hins=None):
                lj = L[j]
                t_j = tp.tile([P, lj, H], bf, tag="t")
                for gi, (n0, g) in enumerate(groups(lj)):
                    if hins is not None:
                        hin = hins[gi]
                    else:
                        hin = hnin.tile([P, 2, 2, g, P], fp8, tag="hnin")
                        nc.sync.dma_start(
                            out=hin,
                            in_=hnf_re[:, :, :, OFF[j] + n0:OFF[j] + n0 + g, :])
                    for k in range(g):
                        s1_n(j, n0 + k, zb, t_j, hin, k)
                return t_j

            def newton(j):
                lj = L[j]
                sm = sumt[:, j, :lj]
                sq_ = sumsq[:, j, :lj]
                varH = nwt.tile([P, N], f32, tag="n_varH")
                a_t = nwt.tile([P, N], f32, tag="n_a")
                y_t = nwt.tile([P, N], f32, tag="n_y")
                u_t = nwt.tile([P, N], f32, tag="n_u")
                w_t = nwt.tile([P, N], f32, tag="n_w")
                musq = nwt.tile([P, N], f32, tag="n_musq")
                varH, a_t, y_t = varH[:, :lj], a_t[:, :lj], y_t[:, :lj]
                u_t, w_t, musq = u_t[:, :lj], w_t[:, :lj], musq[:, :lj]
                nc.vector.tensor_mul(musq, sm, sm)
                nc.vector.scalar_tensor_tensor(
                    out=varH, in0=musq, scalar=-1.0 / H, in1=sq_,
                    op0=Alu.mult, op1=Alu.add)
                nc.vector.tensor_scalar(out=a_t, in0=varH, scalar1=1.0 / H,
                                        scalar2=LN_EPS, op0=Alu.mult,
                                        op1=Alu.add)
                nc.vector.tensor_scalar(out=y_t, in0=a_t, scalar1=-1.35,
                                        scalar2=2.20, op0=Alu.mult,
                                        op1=Alu.add)
                nc.vector.tensor_scalar_max(y_t, y_t, 0.15)
                for _ in range(3):
                    nc.vector.tensor_mul(u_t, a_t, y_t)
                    nc.vector.tensor_mul(u_t, u_t, y_t)
                    nc.vector.tensor_scalar(out=w_t, in0=u_t, scalar1=-0.5,
                                            scalar2=1.5, op0=Alu.mult,
                                            op1=Alu.add)
                    nc.vector.tensor_mul(y_t, y_t, w_t)
                nc.vector.tensor_copy(rs_all[:, j, :lj], y_t)
                nc.vector.tensor_mul(u_t, sm, y_t)
                nc.vector.tensor_scalar_mul(nmurs[:, j, :lj], u_t, -1.0 / H)

            def s2a_n(j, n, qs, t_j, hf_j, sc_j):
                hn = hnp.tile([P, H], bf, tag="hn")
                nc.scalar.activation(out=hn, in_=t_j[:, n],
                                     func=Act.Gelu,
                                     scale=rs_all[:, j, n:n + 1],
                                     bias=nmurs[:, j, n:n + 1])
                tps = trps.tile([P, 4, P], bf, tag="tr")
                for fc in range(4):
                    nc.tensor.transpose(tps[:, fc],
                                        hn[:, fc * P:(fc + 1) * P],
                                        identb)
                nc.scalar.copy(out=hf_j[:, n], in_=tps)
                kps = mmps.tile([P, H], f32, tag="mm")
                for fc in range(4):
                    nc.tensor.matmul(kps, hf_j[:, n, fc], wk_t[:, fc],
                                     start=fc == 0, stop=fc == 3)
                pkt = pkp.tile([P, H], f32, tag="pk")
                nc.vector.tensor_mul(pkt, kps, qs)
                nc.vector.reduce_sum(
                    out=sc_j[:, :, n],
                    in_=pkt.rearrange("c (h d) -> c h d", h=NHEAD),
                    axis=X)

            def softmax(j, sc_j):
                # caller groups calls so Exp excursions share one table load
                lj = L[j]
                e_j = smx.tile([P, NHEAD, N], f32, tag="e")
                at_j = smx.tile([P, NHEAD, N], f32, tag="at")
                ssum = smx.tile([P, NHEAD], f32, tag="ssum")
                e_j, at_j = e_j[:, :, :lj], at_j[:, :, :lj]
                nc.vector.tensor_add(
                    e_j, sc_j[:, :, :lj],
                    mb_t[:, None, OFF[j]:OFF[j] + lj].to_broadcast(
                        (P, NHEAD, lj)))
                nc.scalar.activation(out=e_j, in_=e_j, func=Act.Exp)
                nc.vector.reduce_sum(out=ssum, in_=e_j, axis=X)
                nc.vector.reciprocal(ssum, ssum)
                nc.vector.tensor_mul(
                    at_j, e_j,
                    ssum[:, :, None].to_broadcast((P, NHEAD, lj)))
                return at_j

            def s3_n(j, n, hf_j, at_j, ctx):
                lj = L[j]
                vps = mmps.tile([P, H], f32, tag="mm")
                for fc in range(4):
                    nc.tensor.matmul(vps, hf_j[:, n, fc], wv_t[:, fc],
                                     start=fc == 0, stop=fc == 3)
                pvt = pvp.tile([P, H], bf, tag="pv")
                nc.vector.tensor_mul(
                    pvt.rearrange("c (h d) -> c h d", h=NHEAD),
                    vps.rearrange("c (h d) -> c h d", h=NHEAD),
                    at_j[:, :, n:n + 1].to_broadcast((P, NHEAD, DH)))
                nc.tensor.matmul(ctx, identb, pvt,
                                 start=n == 0, stop=n == lj - 1)

            def s3_tail(j, ctx):
                cs = tail.tile([P, H], bf, tag="cs")
                nc.vector.tensor_copy(out=cs, in_=ctx)
                cts = trps.tile([P, 4, P], bf, tag="tr")
                for fc in range(4):
                    nc.tensor.transpose(cts[:, fc], cs[:, fc * P:(fc + 1) * P],
                                        identb)
                ctf = tail.tile([P, 4, P], bf, tag="ctf")
                nc.vector.tensor_copy(out=ctf, in_=cts)
                ops = mmps.tile([P, H], f32, tag="mm")
                for fc in range(4):
                    nc.tensor.matmul(ops, ctf[:, fc], wo_t[:, fc],
                                     start=fc == 0, stop=fc == 3)
                ot = tail.tile([P, H], bf, tag="ot")
                nc.vector.tensor_copy(out=ot, in_=ops)
                nc.sync.dma_start(out=out_d[j * P:(j + 1) * P, :], in_=ot)

            # ---- interleaved 3-deep software pipeline ----
            # Engines drain their queues in program order, so per-neighbor
            # emission round-robins the three live stages: attention scores
            # for slot j, projection (stage1) for slot j+1, and the V/ctx
            # accumulation for slot j-1.  Each engine's queue then always
            # holds ready work.
            hins0 = issue_hin(0)
            w1s_t = singles.tile([P, 2, 2, H], fp8, tag="w1s8")
            nc.sync.dma_start(
                out=w1s_t,
                in_=w1s_d.rearrange("(q two p) j -> p q two j", q=2, two=2, p=P))
            hcf_t = singles.tile([P, 4, BL], bf, tag="hcf")
            nc.sync.dma_start(out=hcf_t,
                              in_=hcf_d.rearrange("(fc p) c -> p fc c", p=P))
            w1b_t = load_w(w1b_d)
            wq_t = load_w(wq_d)
            zb_c, qs_c = zq(0)
            t_c = stage1(0, zb_c, hins0)
            wk_t = load_w(wk_d)
            wv_t = load_w(wv_d)
            wo_t = load_w(wo_d)
            mb_t = singles.tile([P, sum(L)], f32, tag="mb")
            nc.sync.dma_start(out=mb_t, in_=mb_d[:, :])
            newton(0)

            prev = None  # (j-1, hf, at) awaiting its stage3
            for j in range(NS):
                lj = L[j]
                hf_j = stash.tile([P, lj, 4, P], bf, tag="hf_stash")
                sc_j = smx.tile([P, NHEAD, N], f32, tag="sc")
                if j + 1 < NS:
                    zb_n, qs_n = zq(j + 1)
                    lnx = L[j + 1]
                    t_n = tp.tile([P, lnx, H], bf, tag="t")
                    gplan = groups(lnx)
                else:
                    lnx, gplan = 0, []
                if prev is not None:
                    pj, phf, pat = prev
                    ctx = ctxps.tile([P, H], f32, tag="ctx")
                gi = 0
                hin = None
                for n in range(max(lj, lnx, L[prev[0]] if prev else 0)):
                    # stage3 of j-1 first: its inputs are all ready, so the
                    # in-order engine queues keep draining while newton(j)
                    # (the gelu prerequisite) finishes on Pool.
                    if prev is not None and n < L[pj]:
                        s3_n(pj, n, phf, pat, ctx)
                    if n < lj:
                        s2a_n(j, n, qs_c, t_c, hf_j, sc_j)
                    if n < lnx:
                        n0, g = gplan[gi] if gi < len(gplan) else (None, 0)
                        if n0 == n:
                            hin = hnin.tile([P, 2, 2, g, P], fp8, tag="hnin")
                            nc.sync.dma_start(
                                out=hin,
                                in_=hnf_re[:, :, :,
                                           OFF[j + 1] + n0:OFF[j + 1] + n0 + g, :])
                            gi += 1
                            goff = n0
                        s1_n(j + 1, n, zb_n, t_n, hin, n - goff)
                if prev is not None:
                    s3_tail(pj, ctx)
                if j + 1 < NS:
                    newton(j + 1)
                at_j = softmax(j, sc_j)
                prev = (j, hf_j, at_j)
                t_c, qs_c = (t_n, qs_n) if j + 1 < NS else (None, None)

            pj, phf, pat = prev
            ctx = ctxps.tile([P, H], f32, tag="ctx")
            for n in range(L[pj]):
                s3_n(pj, n, phf, pat, ctx)
            s3_tail(pj, ctx)

    nc.finalize()
    return nc


def _get_nc(L=None):
    key = ("nc", tuple(L) if L else None, tuple(sorted(_OPTS.items())))
    if key not in _CACHE:
        assert L is not None
        _CACHE[key] = _build_nc(list(L))
    return _CACHE[key]


def _pack_inputs(h_center, h_neighbors, neighbor_mask, W1, Wq, Wk, Wv, Wo):
    hn = np.asarray(h_neighbors, np.float32)
    hc = np.asarray(h_center, np.float32)
    mask = np.asarray(neighbor_mask)
    W1 = np.asarray(W1, np.float32)
    w1s = (W1[:H] + W1[H:]).astype(f8)
    w1b = W1[H:].astype(bf16)
    wq = (np.asarray(Wq, np.float32) / np.sqrt(DH)).astype(bf16)
    wk = np.asarray(Wk, bf16)
    wv = np.asarray(Wv, bf16)
    wo = np.asarray(Wo, bf16)

    order, L = _plan(mask)
    SL = sum(L)
    OFF = np.cumsum([0] + list(L))

    # per (core, slot): gather the 128 sorted centers, compact neighbors
    in_maps = []
    for m in range(M):
        hnf = np.zeros((H, SL, P), f8)
        hcf = np.empty((H, BL), np.float32)
        mb = np.full((P, SL), NEG, np.float32)
        for j in range(NS):
            gidx = order[(8 * j + m) * P:(8 * j + m) * P + P]
            hcf[:, j * P:(j + 1) * P] = hc[gidx].T
            for c, g in enumerate(gidx):
                act = np.nonzero(mask[g])[0]
                k = len(act)
                # [k, H] -> [H, k]
                hnf[:, OFF[j]:OFF[j] + k, c] = hn[g, act, :].T.astype(f8)
                mb[c, OFF[j]:OFF[j] + k] = 0.0
        in_maps.append({
            "hnf": np.ascontiguousarray(hnf),
            "hcf": np.ascontiguousarray(hcf.astype(bf16)),
            "mb": np.ascontiguousarray(mb),
            "w1s": w1s, "w1b": w1b, "wq": wq, "wk": wk, "wv": wv, "wo": wo,
        })
    return in_maps, order, L


def _fast_path_ok(b1, ln_g, ln_b, bq, bk, bv, bo):
    return (np.all(np.asarray(b1) == 0) and np.all(np.asarray(ln_g) == 1)
            and np.all(np.asarray(ln_b) == 0) and np.all(np.asarray(bq) == 0)
            and np.all(np.asarray(bv) == 0) and np.all(np.asarray(bo) == 0))


def _np_fallback(h_center, h_neighbors, W1, b1, ln_g, ln_b, Wq, bq, Wk, bk,
                 Wv, bv, Wo, bo, neighbor_mask):
    from scipy.special import erf

    hc = np.asarray(h_center, np.float32)
    hn = np.asarray(h_neighbors, np.float32)
    diff = hn - hc[:, None, :]
    comb = np.concatenate([hn, diff], -1)
    pre = comb @ W1 + b1
    mu = pre.mean(-1, keepdims=True)
    var = ((pre - mu) ** 2).mean(-1, keepdims=True)
    x = (pre - mu) / np.sqrt(var + LN_EPS) * ln_g + ln_b
    hnp_ = 0.5 * x * (1 + erf(x / np.sqrt(2)))
    Q = (hc @ Wq + bq).reshape(B, NHEAD, DH)
    K = (hnp_ @ Wk + bk).reshape(B, N, NHEAD, DH)
    V = (hnp_ @ Wv + bv).reshape(B, N, NHEAD, DH)
    sc = np.einsum("bhd,bnhd->bhn", Q, K) / np.sqrt(DH)
    sc = np.where(neighbor_mask[:, None, :], sc, -np.inf)
    sc = sc - sc.max(-1, keepdims=True)
    e = np.exp(sc)
    attn = e / e.sum(-1, keepdims=True)
    ctx = np.einsum("bhn,bnhd->bhd", attn, V).reshape(B, H)
    return (ctx @ Wo + bo).astype(np.float32)


def run_spmd(in_maps, L, **kwargs):
    from concourse.bass_utils import run_bass_kernel_spmd

    return run_bass_kernel_spmd(_get_nc(L), in_maps, core_ids=list(range(M)),
                                **kwargs)


def kernel(h_center, h_neighbors, W1, b1, ln_g, ln_b, Wq, bq, Wk, bk, Wv, bv,
           Wo, bo, neighbor_mask):
    if not _fast_path_ok(b1, ln_g, ln_b, bq, bk, bv, bo):
        return _np_fallback(h_center, h_neighbors, W1, b1, ln_g, ln_b, Wq, bq,
                            Wk, bk, Wv, bv, Wo, bo, neighbor_mask)
    in_maps, order, L = _pack_inputs(h_center, h_neighbors, neighbor_mask, W1,
                                     Wq, Wk, Wv, Wo)
    res = run_spmd(in_maps, L)
    dev_rows = np.concatenate(
        [np.asarray(r["out"], np.float32) for r in res.results], axis=0)
    # device row m*BL + j*P + c holds original center order[(8j+m)*P + c]
    out = np.empty((B, H), np.float32)
    out[_dev_perm(order)] = dev_rows
    return out


def _dev_perm(order):
    idx = np.empty(B, np.int64)
    for m in range(M):
        for j in range(NS):
            idx[m * BL + j * P:m * BL + (j + 1) * P] = \
                order[(8 * j + m) * P:(8 * j + m + 1) * P]
    return idx


# revision 5
# speedup vs baseline: 2.9415x; 1.2670x over previous
"""DifferenceAwareAggregator v2 — Bass/Tile kernel, data-parallel on 8 cores.

Changes vs v1:
  * Neighbor COMPACTION: ~50% of neighbors are masked out; the host sorts
    centers by active-neighbor count, deals sorted 128-center tiles
    round-robin to cores (so all cores share one NEFF with identical
    per-slot neighbor counts L[j] = band max), and compacts each center's
    active neighbors to the front.  All per-neighbor work (pre/LN/Gelu/
    transpose/K/V/scores) shrinks to sum(L)/256 ~= 55%.
  * pre matmul in fp8e4 DoubleRow (2 contraction tiles per instruction):
    hnf + w1s shipped as fp8; zb/Q/K/V stay bf16 (V-fp8 fails the 2e-2
    gate; measured rel err with pre-fp8 is ~1.47e-2).
  * Interleaved 3-deep software pipeline: engines drain queues in program
    order, so per-neighbor emission round-robins scores(j) / projection
    (j+1) / V-accumulation(j-1) to keep every queue stocked with ready
    work.  PSUM: 4 matmul bufs + 3 transpose bufs + 1 ctx buf.

Algebra: concat([h_n, h_n - h_c]) @ W1 == h_n @ (W1top+W1bot) - h_c @ W1bot.
bk drops out of softmax (per-(b,h) constant shift). 1/sqrt(d) folded into Wq.
"""

import sys

import numpy as np
import ml_dtypes

_TRN = "/opt/trn_rl_repo"
if _TRN not in sys.path:
    sys.path.insert(0, _TRN)

bf16 = ml_dtypes.bfloat16
f8 = ml_dtypes.float8_e4m3fn

M = 8          # cores
B = 8192
N = 32         # max neighbors
H = 512
BL = B // M    # centers per core
P = 128        # partitions
NS = BL // P   # center tiles (slots) per core
NHEAD = 8
DH = H // NHEAD
NEG = -30000.0
LN_EPS = 1e-5

_CACHE: dict = {}
# build-time tuning knobs (sim experiments); key them into the nc cache
_OPTS = {"hf_dve": False, "trps_bufs": 3, "hnp_bufs": 3, "mmps_bufs": 4,
         "ctx_bufs": 1, "pkp_bufs": 3, "pvp_bufs": 3}


def _plan(mask):
    """Sort centers by active count; deal sorted tiles round-robin to cores.

    Returns (order, L) where order[(8j+m)*128 + c] is the original center
    index at core m, slot j, lane c, and L[j] is slot j's neighbor count
    (max over its band of 8 tiles = count of the band's last center).
    """
    k = mask.sum(1).astype(np.int64)
    order = np.argsort(k, kind="stable")
    L = [int(k[order[(8 * j + 8) * P - 1]]) for j in range(NS)]
    return order, L


def _build_nc(L):
    import concourse.mybir as mybir
    import concourse.tile as tile
    from concourse import bacc
    from concourse.masks import make_identity

    f32 = mybir.dt.float32
    bf = mybir.dt.bfloat16
    fp8 = mybir.dt.float8e4
    Alu = mybir.AluOpType
    Act = mybir.ActivationFunctionType
    X = mybir.AxisListType.X
    DR = mybir.MatmulPerfMode.DoubleRow

    SL = sum(L)
    OFF = np.cumsum([0] + list(L))  # slot offsets into the packed n axis

    nc = bacc.Bacc()
    hnf_d = nc.dram_tensor("hnf", [H, SL, P], fp8, kind="ExternalInput")
    hcf_d = nc.dram_tensor("hcf", [H, BL], bf, kind="ExternalInput")
    mb_d = nc.dram_tensor("mb", [P, SL], f32, kind="ExternalInput")
    w1s_d = nc.dram_tensor("w1s", [H, H], fp8, kind="ExternalInput")
    w1b_d = nc.dram_tensor("w1b", [H, H], bf, kind="ExternalInput")
    wq_d = nc.dram_tensor("wq", [H, H], bf, kind="ExternalInput")
    wk_d = nc.dram_tensor("wk", [H, H], bf, kind="ExternalInput")
    wv_d = nc.dram_tensor("wv", [H, H], bf, kind="ExternalInput")
    wo_d = nc.dram_tensor("wo", [H, H], bf, kind="ExternalInput")
    out_d = nc.dram_tensor("out", [BL, H], bf, kind="ExternalOutput")

    # j = q*256 + two*128 + p  (DoubleRow pairs along the contraction dim)
    hnf_re = hnf_d.rearrange("(q two p) n c -> p q two n c", q=2, two=2, p=P)

    from contextlib import ExitStack

    with tile.TileContext(nc) as tc:
        with ExitStack() as stack:
            ec = stack.enter_context
            singles = ec(tc.tile_pool(name="singles", bufs=1))
            zbp = ec(tc.tile_pool(name="zb", bufs=2))
            hnin = ec(tc.tile_pool(name="hnin", bufs=5))
            tp = ec(tc.tile_pool(name="tp", bufs=2))
            sqs = ec(tc.tile_pool(name="sqs", bufs=2))
            hnp = ec(tc.tile_pool(name="hnp", bufs=_OPTS["hnp_bufs"]))
            stash = ec(tc.tile_pool(name="stash", bufs=2))
            pkp = ec(tc.tile_pool(name="pk", bufs=_OPTS["pkp_bufs"]))
            nwt = ec(tc.tile_pool(name="nwt", bufs=2))
            smx = ec(tc.tile_pool(name="smx", bufs=2))
            pvp = ec(tc.tile_pool(name="pv", bufs=_OPTS["pvp_bufs"]))
            tail = ec(tc.tile_pool(name="tail", bufs=2))
            mmps = ec(tc.tile_pool(name="mmps", bufs=_OPTS["mmps_bufs"], space="PSUM"))
            trps = ec(tc.tile_pool(name="trps", bufs=_OPTS["trps_bufs"], space="PSUM"))
            ctxps = ec(tc.tile_pool(name="ctxps", bufs=_OPTS["ctx_bufs"], space="PSUM"))

            # ---- persistent staging ----
            def load_w(dram_t):
                t = singles.tile([P, 4, H], bf, tag=f"w_{dram_t.name}")
                nc.sync.dma_start(out=t,
                                  in_=dram_t.rearrange("(fc p) j -> p fc j", p=P))
                return t

            hcf_t = w1s_t = w1b_t = wq_t = None
            wk_t = wv_t = wo_t = mb_t = None

            identb = singles.tile([P, P], bf, tag="identb")
            make_identity(nc, identb)

            sumt = singles.tile([P, NS, N], f32, tag="sumt")
            sumsq = singles.tile([P, NS, N], f32, tag="sumsq")
            rs_all = singles.tile([P, NS, N], f32, tag="rs_all")
            nmurs = singles.tile([P, NS, N], f32, tag="nmurs")

            def groups(lj):
                g, n0 = [], 0
                while n0 < lj:
                    g.append((n0, min(4, lj - n0)))
                    n0 += min(4, lj - n0)
                return g

            def zq(j):
                zps = mmps.tile([P, H], f32, tag="mm")
                for fc in range(4):
                    nc.tensor.matmul(zps, hcf_t[:, fc, j * P:(j + 1) * P],
                                     w1b_t[:, fc], start=fc == 0, stop=fc == 3)
                zb = zbp.tile([P, H], f32, tag="zb")
                nc.scalar.copy(out=zb, in_=zps)
                qps = mmps.tile([P, H], f32, tag="mm")
                for fc in range(4):
                    nc.tensor.matmul(qps, hcf_t[:, fc, j * P:(j + 1) * P],
                                     wq_t[:, fc], start=fc == 0, stop=fc == 3)
                qs = zbp.tile([P, H], f32, tag="qs")
                nc.scalar.copy(out=qs, in_=qps)
                return zb, qs

            def issue_hin(j):
                tiles = []
                for n0, g in groups(L[j]):
                    hin = hnin.tile([P, 2, 2, g, P], fp8, tag="hnin")
                    nc.sync.dma_start(
                        out=hin,
                        in_=hnf_re[:, :, :, OFF[j] + n0:OFF[j] + n0 + g, :])
                    tiles.append(hin)
                return tiles

            def s1_n(j, n, zb, t_j, hin, k):
                pre = mmps.tile([P, H], f32, tag="mm")
                for hh in range(2):
                    for q in range(2):
                        nc.tensor.matmul(
                            pre[:, hh * 256:(hh + 1) * 256],
                            hin[:, q, :, k, :],
                            w1s_t[:, q, :, hh * 256:(hh + 1) * 256],
                            perf_mode=DR, start=q == 0, stop=q == 1)
                nc.vector.scalar_tensor_tensor(
                    out=t_j[:, n], in0=pre, scalar=0.0, in1=zb,
                    op0=Alu.add, op1=Alu.subtract,
                    accum_out=sumt[:, j, n:n + 1])
                sq = sqs.tile([P, H], bf, tag="sq")
                nc.scalar.activation(out=sq, in_=t_j[:, n],
                                     func=Act.Square,
                                     accum_out=sumsq[:, j, n:n + 1])

            def stage1(j, zb, hins=None):
                lj = L[j]
                t_j = tp.tile([P, lj, H], bf, tag="t")
                for gi, (n0, g) in enumerate(groups(lj)):
                    if hins is not None:
                        hin = hins[gi]
                    else:
                        hin = hnin.tile([P, 2, 2, g, P], fp8, tag="hnin")
                        nc.sync.dma_start(
                            out=hin,
                            in_=hnf_re[:, :, :, OFF[j] + n0:OFF[j] + n0 + g, :])
                    for k in range(g):
                        s1_n(j, n0 + k, zb, t_j, hin, k)
                return t_j

            def newton(j):
                lj = L[j]
                sm = sumt[:, j, :lj]
                sq_ = sumsq[:, j, :lj]
                varH = nwt.tile([P, N], f32, tag="n_varH")
                a_t = nwt.tile([P, N], f32, tag="n_a")
                y_t = nwt.tile([P, N], f32, tag="n_y")
                u_t = nwt.tile([P, N], f32, tag="n_u")
                w_t = nwt.tile([P, N], f32, tag="n_w")
                musq = nwt.tile([P, N], f32, tag="n_musq")
                varH, a_t, y_t = varH[:, :lj], a_t[:, :lj], y_t[:, :lj]
                u_t, w_t, musq = u_t[:, :lj], w_t[:, :lj], musq[:, :lj]
                nc.vector.tensor_mul(musq, sm, sm)
                nc.vector.scalar_tensor_tensor(
                    out=varH, in0=musq, scalar=-1.0 / H, in1=sq_,
                    op0=Alu.mult, op1=Alu.add)
                nc.vector.tensor_scalar(out=a_t, in0=varH, scalar1=1.0 / H,
                                        scalar2=LN_EPS, op0=Alu.mult,
                                        op1=Alu.add)
                nc.vector.tensor_scalar(out=y_t, in0=a_t, scalar1=-1.35,
                                        scalar2=2.20, op0=Alu.mult,
                                        op1=Alu.add)
                nc.vector.tensor_scalar_max(y_t, y_t, 0.15)
                for _ in range(3):
                    nc.vector.tensor_mul(u_t, a_t, y_t)
                    nc.vector.tensor_mul(u_t, u_t, y_t)
                    nc.vector.tensor_scalar(out=w_t, in0=u_t, scalar1=-0.5,
                                            scalar2=1.5, op0=Alu.mult,
                                            op1=Alu.add)
                    nc.vector.tensor_mul(y_t, y_t, w_t)
                nc.vector.tensor_copy(rs_all[:, j, :lj], y_t)
                nc.vector.tensor_mul(u_t, sm, y_t)
                nc.vector.tensor_scalar_mul(nmurs[:, j, :lj], u_t, -1.0 / H)

            def s2a_n(j, n, qs, t_j, hf_j, sc_j):
                hn = hnp.tile([P, H], bf, tag="hn")
                nc.scalar.activation(out=hn, in_=t_j[:, n],
                                     func=Act.Gelu,
                                     scale=rs_all[:, j, n:n + 1],
                                     bias=nmurs[:, j, n:n + 1])
                tps = trps.tile([P, 4, P], bf, tag="tr")
                for fc in range(4):
                    nc.tensor.transpose(tps[:, fc],
                                        hn[:, fc * P:(fc + 1) * P],
                                        identb)
                nc.scalar.copy(out=hf_j[:, n], in_=tps)
                kps = mmps.tile([P, H], f32, tag="mm")
                for fc in range(4):
                    nc.tensor.matmul(kps, hf_j[:, n, fc], wk_t[:, fc],
                                     start=fc == 0, stop=fc == 3)
                pkt = pkp.tile([P, H], f32, tag="pk")
                nc.vector.tensor_mul(pkt, kps, qs)
                nc.vector.reduce_sum(
                    out=sc_j[:, :, n],
                    in_=pkt.rearrange("c (h d) -> c h d", h=NHEAD),
                    axis=X)

            def softmax(j, sc_j):
                # caller groups calls so Exp excursions share one table load
                lj = L[j]
                e_j = smx.tile([P, NHEAD, N], f32, tag="e")
                at_j = smx.tile([P, NHEAD, N], f32, tag="at")
                ssum = smx.tile([P, NHEAD], f32, tag="ssum")
                e_j, at_j = e_j[:, :, :lj], at_j[:, :, :lj]
                nc.vector.tensor_add(
                    e_j, sc_j[:, :, :lj],
                    mb_t[:, None, OFF[j]:OFF[j] + lj].to_broadcast(
                        (P, NHEAD, lj)))
                nc.scalar.activation(out=e_j, in_=e_j, func=Act.Exp)
                nc.vector.reduce_sum(out=ssum, in_=e_j, axis=X)
                nc.vector.reciprocal(ssum, ssum)
                nc.vector.tensor_mul(
                    at_j, e_j,
                    ssum[:, :, None].to_broadcast((P, NHEAD, lj)))
                return at_j

            def s3_n(j, n, hf_j, at_j, ctx):
                lj = L[j]
                vps = mmps.tile([P, H], f32, tag="mm")
                for fc in range(4):
                    nc.tensor.matmul(vps, hf_j[:, n, fc], wv_t[:, fc],
                                     start=fc == 0, stop=fc == 3)
                pvt = pvp.tile([P, H], bf, tag="pv")
                nc.vector.tensor_mul(
                    pvt.rearrange("c (h d) -> c h d", h=NHEAD),
                    vps.rearrange("c (h d) -> c h d", h=NHEAD),
                    at_j[:, :, n:n + 1].to_broadcast((P, NHEAD, DH)))
                nc.tensor.matmul(ctx, identb, pvt,
                                 start=n == 0, stop=n == lj - 1)

            def s3_tail(j, ctx):
                cs = tail.tile([P, H], bf, tag="cs")
                nc.vector.tensor_copy(out=cs, in_=ctx)
                cts = trps.tile([P, 4, P], bf, tag="tr")
                for fc in range(4):
                    nc.tensor.transpose(cts[:, fc], cs[:, fc * P:(fc + 1) * P],
                                        identb)
                ctf = tail.tile([P, 4, P], bf, tag="ctf")
                nc.vector.tensor_copy(out=ctf, in_=cts)
                ops = mmps.tile([P, H], f32, tag="mm")
                for fc in range(4):
                    nc.tensor.matmul(ops, ctf[:, fc], wo_t[:, fc],
                                     start=fc == 0, stop=fc == 3)
                ot = tail.tile([P, H], bf, tag="ot")
                nc.vector.tensor_copy(out=ot, in_=ops)
                nc.sync.dma_start(out=out_d[j * P:(j + 1) * P, :], in_=ot)

            # ---- interleaved 3-deep software pipeline ----
            # Engines drain their queues in program order, so per-neighbor
            # emission round-robins the three live stages: attention scores
            # for slot j, projection (stage1) for slot j+1, and the V/ctx
            # accumulation for slot j-1.  Each engine's queue then always
            # holds ready work.
            hins0 = issue_hin(0)
            w1s_t = singles.tile([P, 2, 2, H], fp8, tag="w1s8")
            nc.sync.dma_start(
                out=w1s_t,
                in_=w1s_d.rearrange("(q two p) j -> p q two j", q=2, two=2, p=P))
            hcf_t = singles.tile([P, 4, BL], bf, tag="hcf")
            nc.sync.dma_start(out=hcf_t,
                              in_=hcf_d.rearrange("(fc p) c -> p fc c", p=P))
            w1b_t = load_w(w1b_d)
            wq_t = load_w(wq_d)
            zb_c, qs_c = zq(0)
            t_c = stage1(0, zb_c, hins0)
            wk_t = load_w(wk_d)
            wv_t = load_w(wv_d)
            wo_t = load_w(wo_d)
            mb_t = singles.tile([P, sum(L)], f32, tag="mb")
            nc.sync.dma_start(out=mb_t, in_=mb_d[:, :])
            newton(0)

            prev = None  # (j-1, hf, at) awaiting its stage3
            for j in range(NS):
                lj = L[j]
                hf_j = stash.tile([P, lj, 4, P], bf, tag="hf_stash")
                sc_j = smx.tile([P, NHEAD, N], f32, tag="sc")
                if j + 1 < NS:
                    zb_n, qs_n = zq(j + 1)
                    lnx = L[j + 1]
                    t_n = tp.tile([P, lnx, H], bf, tag="t")
                    gplan = groups(lnx)
                else:
                    lnx, gplan = 0, []
                if prev is not None:
                    pj, phf, pat = prev
                    ctx = ctxps.tile([P, H], f32, tag="ctx")
                gi = 0
                hin = None
                for n in range(max(lj, lnx, L[prev[0]] if prev else 0)):
                    # stage3 of j-1 first: its inputs are all ready, so the
                    # in-order engine queues keep draining while newton(j)
                    # (the gelu prerequisite) finishes on Pool.
                    if prev is not None and n < L[pj]:
                        s3_n(pj, n, phf, pat, ctx)
                    if n < lj:
                        s2a_n(j, n, qs_c, t_c, hf_j, sc_j)
                    if n < lnx:
                        n0, g = gplan[gi] if gi < len(gplan) else (None, 0)
                        if n0 == n:
                            hin = hnin.tile([P, 2, 2, g, P], fp8, tag="hnin")
                            nc.sync.dma_start(
                                out=hin,
                                in_=hnf_re[:, :, :,
                                           OFF[j + 1] + n0:OFF[j + 1] + n0 + g, :])
                            gi += 1
                            goff = n0
                        s1_n(j + 1, n, zb_n, t_n, hin, n - goff)
                if prev is not None:
                    s3_tail(pj, ctx)
                if j + 1 < NS:
                    newton(j + 1)
                at_j = softmax(j, sc_j)
                prev = (j, hf_j, at_j)
                t_c, qs_c = (t_n, qs_n) if j + 1 < NS else (None, None)

            pj, phf, pat = prev
            ctx = ctxps.tile([P, H], f32, tag="ctx")
            for n in range(L[pj]):
                s3_n(pj, n, phf, pat, ctx)
            s3_tail(pj, ctx)

    nc.finalize()
    return nc


def _get_nc(L=None):
    key = ("nc", tuple(L) if L else None, tuple(sorted(_OPTS.items())))
    if key not in _CACHE:
        assert L is not None
        _CACHE[key] = _build_nc(list(L))
    return _CACHE[key]


def _pack_inputs(h_center, h_neighbors, neighbor_mask, W1, Wq, Wk, Wv, Wo):
    hn = np.asarray(h_neighbors, np.float32)
    hc = np.asarray(h_center, np.float32)
    mask = np.asarray(neighbor_mask)
    W1 = np.asarray(W1, np.float32)
    w1s = (W1[:H] + W1[H:]).astype(f8)
    w1b = W1[H:].astype(bf16)
    wq = (np.asarray(Wq, np.float32) / np.sqrt(DH)).astype(bf16)
    wk = np.asarray(Wk, bf16)
    wv = np.asarray(Wv, bf16)
    wo = np.asarray(Wo, bf16)

    order, L = _plan(mask)
    SL = sum(L)
    OFF = np.cumsum([0] + list(L))

    # per (core, slot): gather the 128 sorted centers, compact neighbors
    in_maps = []
    for m in range(M):
        hnf = np.zeros((H, SL, P), f8)
        hcf = np.empty((H, BL), np.float32)
        mb = np.full((P, SL), NEG, np.float32)
        for j in range(NS):
            gidx = order[(8 * j + m) * P:(8 * j + m) * P + P]
            hcf[:, j * P:(j + 1) * P] = hc[gidx].T
            for c, g in enumerate(gidx):
                act = np.nonzero(mask[g])[0]
                k = len(act)
                # [k, H] -> [H, k]
                hnf[:, OFF[j]:OFF[j] + k, c] = hn[g, act, :].T.astype(f8)
                mb[c, OFF[j]:OFF[j] + k] = 0.0
        in_maps.append({
            "hnf": np.ascontiguousarray(hnf),
            "hcf": np.ascontiguousarray(hcf.astype(bf16)),
            "mb": np.ascontiguousarray(mb),
            "w1s": w1s, "w1b": w1b, "wq": wq, "wk": wk, "wv": wv, "wo": wo,
        })
    return in_maps, order, L


def _fast_path_ok(b1, ln_g, ln_b, bq, bk, bv, bo):
    return (np.all(np.asarray(b1) == 0) and np.all(np.asarray(ln_g) == 1)
            and np.all(np.asarray(ln_b) == 0) and np.all(np.asarray(bq) == 0)
            and np.all(np.asarray(bv) == 0) and np.all(np.asarray(bo) == 0))


def _np_fallback(h_center, h_neighbors, W1, b1, ln_g, ln_b, Wq, bq, Wk, bk,
                 Wv, bv, Wo, bo, neighbor_mask):
    from scipy.special import erf

    hc = np.asarray(h_center, np.float32)
    hn = np.asarray(h_neighbors, np.float32)
    diff = hn - hc[:, None, :]
    comb = np.concatenate([hn, diff], -1)
    pre = comb @ W1 + b1
    mu = pre.mean(-1, keepdims=True)
    var = ((pre - mu) ** 2).mean(-1, keepdims=True)
    x = (pre - mu) / np.sqrt(var + LN_EPS) * ln_g + ln_b
    hnp_ = 0.5 * x * (1 + erf(x / np.sqrt(2)))
    Q = (hc @ Wq + bq).reshape(B, NHEAD, DH)
    K = (hnp_ @ Wk + bk).reshape(B, N, NHEAD, DH)
    V = (hnp_ @ Wv + bv).reshape(B, N, NHEAD, DH)
    sc = np.einsum("bhd,bnhd->bhn", Q, K) / np.sqrt(DH)
    sc = np.where(neighbor_mask[:, None, :], sc, -np.inf)
    sc = sc - sc.max(-1, keepdims=True)
    e = np.exp(sc)
    attn = e / e.sum(-1, keepdims=True)
    ctx = np.einsum("bhn,bnhd->bhd", attn, V).reshape(B, H)
    return (ctx @ Wo + bo).astype(np.float32)


def run_spmd(in_maps, L, **kwargs):
    from concourse.bass_utils import run_bass_kernel_spmd

    return run_bass_kernel_spmd(_get_nc(L), in_maps, core_ids=list(range(M)),
                                **kwargs)


def kernel(h_center, h_neighbors, W1, b1, ln_g, ln_b, Wq, bq, Wk, bk, Wv, bv,
           Wo, bo, neighbor_mask):
    if not _fast_path_ok(b1, ln_g, ln_b, bq, bk, bv, bo):
        return _np_fallback(h_center, h_neighbors, W1, b1, ln_g, ln_b, Wq, bq,
                            Wk, bk, Wv, bv, Wo, bo, neighbor_mask)
    in_maps, order, L = _pack_inputs(h_center, h_neighbors, neighbor_mask, W1,
                                     Wq, Wk, Wv, Wo)
    res = run_spmd(in_maps, L)
    dev_rows = np.concatenate(
        [np.asarray(r["out"], np.float32) for r in res.results], axis=0)
    # device row m*BL + j*P + c holds original center order[(8j+m)*P + c]
    out = np.empty((B, H), np.float32)
    out[_dev_perm(order)] = dev_rows
    return out


def _dev_perm(order):
    idx = np.empty(B, np.int64)
    for m in range(M):
        for j in range(NS):
            idx[m * BL + j * P:m * BL + (j + 1) * P] = \
                order[(8 * j + m) * P:(8 * j + m + 1) * P]
    return idx
